# revision 6
# baseline (speedup 1.0000x reference)
"""AuthPct metric kernel for 8 Trainium2 NeuronCores.

Strategy (per sharding hint): shard real_stats rows across the 8 cores
(1536 rows each).  Each core computes, for every column feature f_j
(all 12288 reals and all 12288 gens), the quantity

    X[j, i] = 2 * f_j . r_i  -  |r_i|^2      (i in the core's row shard)

via PE matmuls: lhsT = column features (128-col j-tiles), rhs = 2*r_i^T
(resident shard), plus one K=2 augmented matmul whose rhs rows are
-hi/lo split of |r_i|^2 (exact under reduced-precision f32r input
rounding).  Then  dist^2(j, i) = |f_j|^2 - X[j, i],  so

    min_i dist^2 = |f_j|^2 - max_i X[j, i]

The per-core partial max over i (top-8 via DVE `max`, plus `max_index`
for gen argmin payloads) is written out; the host combines the 8 cores'
partials, resolves the real-diagonal (a core's own j==i entry shows up
as X == |r_j|^2, ~300 above any true neighbor, so top-2 is used there),
gathers d2 = realNN[argmin], applies sigmoid and the mean.

Matmuls run in float32r (full-rate fp32 mode with reduced-precision
multiplies); reductions in exact fp32.
"""

import numpy as np

N = 12288
D = 256
NCORES = 8
SHARD = N // NCORES          # 1536 rows per core
JTILE = 128                  # j columns per tile (PSUM partitions)
NJT = N // JTILE             # 96 j-tiles per distance matrix
NT = 512                     # i elements per matmul (PSUM bank)
NIT = SHARD // NT            # 3 i-tiles

_cached_nc = None


def _build_nc():
    import concourse.mybir as mybir
    from concourse import bacc
    from concourse.tile import TileContext

    f32 = mybir.dt.float32
    bf16 = mybir.dt.bfloat16
    u32 = mybir.dt.uint32

    nc = bacc.Bacc("TRN2", target_bir_lowering=False, debug=False,
                   num_devices=NCORES)

    colr = nc.dram_tensor("colr", [D, N], bf16, kind="ExternalInput")
    colg = nc.dram_tensor("colg", [D, N], bf16, kind="ExternalInput")
    rhs = nc.dram_tensor("rhs", [D, SHARD], bf16, kind="ExternalInput")
    aug = nc.dram_tensor("aug", [2, SHARD], bf16, kind="ExternalInput")
    ones = nc.dram_tensor("ones", [JTILE, JTILE], bf16, kind="ExternalInput")

    o_realv = nc.dram_tensor("o_realv", [128, NJT * 8], f32,
                             kind="ExternalOutput")
    o_genv = nc.dram_tensor("o_genv", [128, NJT * 8], f32,
                            kind="ExternalOutput")
    o_geni = nc.dram_tensor("o_geni", [128, NJT * 8], u32,
                            kind="ExternalOutput")

    with TileContext(nc) as tc:
        with (
            tc.tile_pool(name="const", bufs=1) as constp,
            tc.tile_pool(name="lhs", bufs=4) as lhsp,
            tc.tile_pool(name="wide", bufs=4) as widep,
            tc.tile_pool(name="outb", bufs=1) as outp,
            tc.tile_pool(name="ps", bufs=8, space="PSUM") as psp,
        ):
            # Resident rhs: both K-chunks of 2*realT shard, [128, 2*1536].
            # Loaded in per-i-tile slices so the first matmul group only
            # waits on the slices it reads (shaves the startup ramp).
            rhs_sb = constp.tile([128, 2 * SHARD], bf16)
            for it in range(NIT):
                io = it * NT
                nc.sync.dma_start(out=rhs_sb[:, io:io + NT],
                                  in_=rhs[0:128, io:io + NT])
                nc.sync.dma_start(out=rhs_sb[:, SHARD + io:SHARD + io + NT],
                                  in_=rhs[128:256, io:io + NT])
            # aug/ones zero-padded to K=128: a K=2 matmul stalls the PE
            # pipeline (~600 ns/MM vs 216); a full-K matmul with zero rows
            # runs at line rate.
            aug_sb = constp.tile([128, SHARD], bf16)
            nc.vector.memset(aug_sb, 0.0)
            nc.sync.dma_start(out=aug_sb[0:2, :], in_=aug[:, :])
            ones_sb = constp.tile([JTILE, JTILE], bf16)
            nc.sync.dma_start(out=ones_sb[:, :], in_=ones[:, :])

            realv = outp.tile([128, NJT * 8], f32)
            genv = outp.tile([128, NJT * 8], f32)
            geni = outp.tile([128, NJT * 8], u32)

            for jt in range(NJT):
                jo = jt * JTILE
                # load both K-chunks of this j-tile's column features
                lhs_r = lhsp.tile([128, 2 * JTILE], bf16, tag="lhs_r")
                nc.sync.dma_start(
                    out=lhs_r[:, :].rearrange("p (c j) -> p c j", c=2),
                    in_=colr[:, jo:jo + JTILE].rearrange(
                        "(c p) j -> p c j", c=2),
                )
                lhs_g = lhsp.tile([128, 2 * JTILE], bf16, tag="lhs_g")
                nc.sync.dma_start(
                    out=lhs_g[:, :].rearrange("p (c j) -> p c j", c=2),
                    in_=colg[:, jo:jo + JTILE].rearrange(
                        "(c p) j -> p c j", c=2),
                )

                wide_r = widep.tile([128, SHARD], f32, tag="wide_r")
                wide_g = widep.tile([128, SHARD], f32, tag="wide_g")

                for it in range(NIT):
                    io = it * NT
                    for dist, lhs_t, wide in (
                        (0, lhs_r, wide_r),
                        (1, lhs_g, wide_g),
                    ):
                        ps = psp.tile([128, NT], f32)
                        nc.tensor.matmul(
                            out=ps[:, :],
                            lhsT=lhs_t[:, 0:JTILE],
                            rhs=rhs_sb[:, io:io + NT],
                            start=True, stop=False,
                        )
                        nc.tensor.matmul(
                            out=ps[:, :],
                            lhsT=lhs_t[:, JTILE:2 * JTILE],
                            rhs=rhs_sb[:, SHARD + io:SHARD + io + NT],
                            start=False, stop=False,
                        )
                        nc.tensor.matmul(
                            out=ps[:, :],
                            lhsT=ones_sb[:, :],
                            rhs=aug_sb[:, io:io + NT],
                            start=False, stop=True,
                        )
                        nc.scalar.activation(
                            out=wide[:, io:io + NT],
                            in_=ps[:, :],
                            func=mybir.ActivationFunctionType.Copy,
                        )

                nc.vector.max(out=realv[:, jt * 8:(jt + 1) * 8],
                              in_=wide_r[:, :])
                nc.vector.max(out=genv[:, jt * 8:(jt + 1) * 8],
                              in_=wide_g[:, :])
                nc.vector.max_index(out=geni[:, jt * 8:(jt + 1) * 8],
                                    in_max=genv[:, jt * 8:(jt + 1) * 8],
                                    in_values=wide_g[:, :])

            nc.sync.dma_start(out=o_realv[:, :], in_=realv[:, :])
            nc.sync.dma_start(out=o_genv[:, :], in_=genv[:, :])
            nc.sync.dma_start(out=o_geni[:, :], in_=geni[:, :])

    nc.compile()
    return nc


def _trunc_hi(x):
    """Keep 10 mantissa bits (safe to represent under any plausible f32r
    input rounding); the residual carries the rest."""
    v = x.astype(np.float32).view(np.uint32) & np.uint32(0xFFFFE000)
    return v.view(np.float32)


def kernel(real_stats, gen_stats, _trace=False):
    import ml_dtypes
    from concourse.bass_utils import run_bass_kernel_spmd

    bf = ml_dtypes.bfloat16
    global _cached_nc
    real = np.ascontiguousarray(np.asarray(real_stats, dtype=np.float32))
    gen = np.ascontiguousarray(np.asarray(gen_stats, dtype=np.float32))

    realT = np.ascontiguousarray(real.T)                  # [D, N]
    genT = np.ascontiguousarray(gen.T)
    colr_bf = realT.astype(bf)
    colg_bf = genT.astype(bf)
    rhs_bf = (2.0 * realT).astype(bf)                     # [D, N]
    b2 = np.sum(real.astype(np.float64) ** 2, axis=1).astype(np.float32)
    a2g = np.sum(gen.astype(np.float64) ** 2, axis=1).astype(np.float32)
    b2hi = b2.astype(bf).astype(np.float32)
    b2lo = (b2 - b2hi).astype(np.float32)
    ones = np.zeros((JTILE, JTILE), dtype=bf)
    ones[0:2, :] = 1

    in_maps = []
    for c in range(NCORES):
        sl = slice(c * SHARD, (c + 1) * SHARD)
        in_maps.append({
            "colr": colr_bf,
            "colg": colg_bf,
            "rhs": np.ascontiguousarray(rhs_bf[:, sl]),
            "aug": np.ascontiguousarray(
                np.stack([-b2hi[sl], -b2lo[sl]])).astype(bf),
            "ones": ones,
        })

    if _cached_nc is None:
        _cached_nc = _build_nc()
    res = run_bass_kernel_spmd(_cached_nc, in_maps,
                               core_ids=list(range(NCORES)),
                               trace=_trace)

    # ---- host combine ----
    def grid(name, c, dtype):
        # [128, NJT*8] -> [N, 8] with j = jt*128 + p
        a = res.results[c][name].reshape(128, NJT, 8)
        return np.ascontiguousarray(a.transpose(1, 0, 2).reshape(N, 8))

    realv = np.stack([grid("o_realv", c, np.float32) for c in range(NCORES)])
    genv = np.stack([grid("o_genv", c, np.float32) for c in range(NCORES)])
    geni = np.stack([grid("o_geni", c, np.uint32) for c in range(NCORES)])

    j = np.arange(N)
    own = j // SHARD                                      # owning core per j
    top1 = realv[:, :, 0]                                 # [8, N]
    own_top1 = top1[own, j]
    own_top2 = realv[own, j, 1]
    # the diagonal entry computes to ~|r_j|^2; real neighbors are ~300 below
    is_diag = np.abs(own_top1 - b2) < 10.0
    own_best = np.where(is_diag, own_top2, own_top1)
    masked = top1.copy()
    masked[own, j] = own_best
    realmax = masked.max(axis=0)
    realNN = np.sqrt(np.maximum(b2 - realmax, 0.0))       # [N]

    gv = genv[:, :, 0]                                    # [8, N]
    cstar = gv.argmax(axis=0)
    d1sq = a2g - gv[cstar, j]
    d1 = np.sqrt(np.maximum(d1sq, 0.0))
    istar = cstar * SHARD + geni[cstar, j, 0]
    d2 = realNN[istar]

    z = (d2 - d1) / 0.1
    authen = np.where(z >= 0, 1.0 / (1.0 + np.exp(-np.abs(z))),
                      np.exp(-np.abs(z)) / (1.0 + np.exp(-np.abs(z))))
    out = -100.0 * np.mean(authen)
    if _trace:
        return np.float32(out), res
    return np.float32(out)
